# revision 1
# baseline (speedup 1.0000x reference)
"""Causal self-attention (B=4, T=2048, C=768, H=12) on 8 NeuronCores.

Sharding: core (b, hg) with b in 0..3, hg in 0..1 handles batch b and head
group hg (6 heads of 64 dims).  Each core computes q/k/v projections for its
head group, causal flash-style attention in transposed-score layout, and its
half of the output projection (rows hg*384..hg*384+384 of w_proj).  The host
sums the two half-projections per batch (the only cross-core reduction).

All matmuls run as float32r (reduced-precision fp32 mode, full PE rate at
N>=256, ~1e-4 relative accuracy).
"""

import numpy as np

import concourse.bass as bass
import concourse.bacc as bacc
import concourse.tile as tile
import concourse.mybir as mybir
from concourse.bass_utils import run_bass_kernel_spmd

F32 = mybir.dt.float32
F32R = mybir.dt.float32r

B, T, C = 4, 2048, 768
H = 12
D = 64
HG = 2            # head groups (cores per batch)
HPG = H // HG     # heads per group = 6
CG = C // HG      # channels per group = 384
KC = C // 128     # contraction chunks for C = 6
TT = T // 128     # t-tiles = 16
QB = T // 512     # query blocks = 4
VW = HPG * (D + 1)  # interleaved v width (ones col per head) = 390

_CACHE = {}


def _build():
    nc = bacc.Bacc("TRN2", target_bir_lowering=False, debug=False, num_devices=8)

    x_ap = nc.dram_tensor("x", [T, C], F32R, kind="ExternalInput").ap()
    wqk_ap = nc.dram_tensor("wqk", [C, 2 * CG], F32R, kind="ExternalInput").ap()
    wv_ap = nc.dram_tensor("wv", [C, VW], F32R, kind="ExternalInput").ap()
    wp_ap = nc.dram_tensor("wp", [CG, C], F32R, kind="ExternalInput").ap()
    bqk_ap = nc.dram_tensor("bqk", [128, KC], F32, kind="ExternalInput").ap()
    bv_ap = nc.dram_tensor("bv", [1, VW], F32R, kind="ExternalInput").ap()
    bp_ap = nc.dram_tensor("bp", [1, C], F32R, kind="ExternalInput").ap()
    ident_ap = nc.dram_tensor("ident", [128, 128], F32R, kind="ExternalInput").ap()
    ones_ap = nc.dram_tensor("ones1", [1, 128], F32R, kind="ExternalInput").ap()
    tri_ap = nc.dram_tensor("tri", [128, 128], F32R, kind="ExternalInput").ap()
    out_ap = nc.dram_tensor("out", [T, C], F32, kind="ExternalOutput").ap()
    rden_dram = nc.dram_tensor("rden_scratch", [HPG * QB, 512], F32).ap()

    with tile.TileContext(nc) as tc:
        import contextlib

        st = contextlib.ExitStack()
        with st:
            const = st.enter_context(tc.tile_pool(name="const", bufs=1))
            big = st.enter_context(tc.tile_pool(name="big", bufs=1))

            # ---- constants / weights
            wqk_sb = const.tile([128, KC, 2 * CG], F32R)
            nc.sync.dma_start(wqk_sb[:], wqk_ap.rearrange("(k p) m -> p k m", p=128))
            wv_sb = const.tile([128, KC, VW], F32R)
            nc.sync.dma_start(wv_sb[:], wv_ap.rearrange("(k p) m -> p k m", p=128))
            wp_sb = const.tile([128, CG // 128, C], F32R)
            nc.sync.dma_start(wp_sb[:], wp_ap.rearrange("(k p) m -> p k m", p=128))
            bqk_sb = const.tile([128, KC], F32)
            nc.sync.dma_start(bqk_sb[:], bqk_ap[:])
            bv_sb = const.tile([1, VW], F32R)
            nc.sync.dma_start(bv_sb[:], bv_ap[:])
            bp_sb = const.tile([1, C], F32R)
            nc.sync.dma_start(bp_sb[:], bp_ap[:])
            ident_sb = const.tile([128, 128], F32R)
            nc.sync.dma_start(ident_sb[:], ident_ap[:])
            ones_sb = const.tile([1, 128], F32R)
            nc.sync.dma_start(ones_sb[:], ones_ap[:])
            tri_sb = const.tile([128, 128], F32R)
            nc.sync.dma_start(tri_sb[:], tri_ap[:])

            # ---- persistent activations
            qkT_sb = big.tile([128, 2 * CG // 128, T], F32R)   # chunks 0-2: q pairs, 3-5: k pairs
            v_sb = big.tile([128, TT, VW], F32R)               # interleaved v + ones cols
            ynT_sb = big.tile([128, CG // 128, T], F32R)       # normalized y^T

            # ================= phase 1: x^T + qkv =================
            with tc.tile_pool(name="xin", bufs=3) as xin_pool, \
                 tc.tile_pool(name="xT", bufs=1) as xT_pool, \
                 tc.tile_pool(name="ps_t", bufs=3, space="PSUM") as ps_t, \
                 tc.tile_pool(name="ps_qk", bufs=2, space="PSUM") as ps_qk, \
                 tc.tile_pool(name="ps_v", bufs=2, space="PSUM") as ps_v:

                xT_sb = xT_pool.tile([128, KC, T], F32R)
                for ti in range(TT):
                    x_t = xin_pool.tile([128, C], F32R)
                    nc.sync.dma_start(x_t[:], x_ap[ti * 128:(ti + 1) * 128, :])
                    for cc in range(KC):
                        pst = ps_t.tile([128, 128], F32)
                        nc.tensor.transpose(
                            pst[:].bitcast(F32R),
                            x_t[:, cc * 128:(cc + 1) * 128],
                            ident_sb[:],
                        )
                        nc.vector.tensor_copy(
                            xT_sb[:, cc, ti * 128:(ti + 1) * 128], pst[:]
                        )

                # q^T and k^T:  psum[m-chunk, t-block] = sum_k w_qk[k,m].T @ x^T[k,t]
                for m in range(2 * CG // 128):
                    for tb in range(QB):
                        pqk = ps_qk.tile([128, 512], F32)
                        for kc in range(KC):
                            nc.tensor.matmul(
                                pqk[:],
                                wqk_sb[:, kc, m * 128:(m + 1) * 128],
                                xT_sb[:, kc, tb * 512:(tb + 1) * 512],
                                start=(kc == 0),
                                stop=(kc == KC - 1),
                            )
                        nc.vector.tensor_scalar_add(
                            qkT_sb[:, m, tb * 512:(tb + 1) * 512],
                            pqk[:],
                            bqk_sb[:, m:m + 1],
                        )

                # v (interleaved, with ones cols from augmented weights/bias)
                for ti in range(TT):
                    pv = ps_v.tile([128, VW], F32)
                    for kc in range(KC):
                        nc.tensor.matmul(
                            pv[:],
                            xT_sb[:, kc, ti * 128:(ti + 1) * 128],
                            wv_sb[:, kc, :],
                            start=(kc == 0),
                            stop=False,
                        )
                    nc.tensor.matmul(
                        pv[:], ones_sb[:], bv_sb[:], start=False, stop=True
                    )
                    nc.vector.tensor_copy(v_sb[:, ti, :], pv[:])

            # ================= phase 2: attention =================
            with tc.tile_pool(name="pt", bufs=3) as pt_pool, \
                 tc.tile_pool(name="den", bufs=4) as den_pool, \
                 tc.tile_pool(name="bc", bufs=3) as bc_pool, \
                 tc.tile_pool(name="shift", bufs=2) as shift_pool, \
                 tc.tile_pool(name="ps_s", bufs=1, space="PSUM") as ps_s, \
                 tc.tile_pool(name="ps_av", bufs=2, space="PSUM") as ps_av, \
                 tc.tile_pool(name="ps_pr", bufs=2, space="PSUM") as ps_pr, \
                 tc.tile_pool(name="out_sb", bufs=2) as out_pool:

                for h in range(HPG):
                    pair, sub = h // 2, h % 2
                    p0 = 64 * sub
                    for qb in range(QB):
                        y_ps = ps_av.tile([65, 512], F32)
                        n_kt = 4 * (qb + 1)
                        for g in range(qb + 1):
                            s_ps = ps_s.tile([128, 2048], F32)
                            diag = (g == qb)
                            pt = pt_pool.tile([128, 2048], F32R)
                            for j in range(4):
                                kt = 4 * g + j
                                off = 128 * j if diag else 0
                                nc.tensor.matmul(
                                    s_ps[:, 512 * j + off:512 * (j + 1)],
                                    qkT_sb[p0:p0 + 64, 3 + pair,
                                           kt * 128:(kt + 1) * 128],
                                    qkT_sb[p0:p0 + 64, pair,
                                           qb * 512 + off:(qb + 1) * 512],
                                    start=True,
                                    stop=True,
                                )
                            if diag:
                                for j in range(4):
                                    off = 128 * j
                                    nc.scalar.activation(
                                        pt[:, 512 * j + off:512 * (j + 1)],
                                        s_ps[:, 512 * j + off:512 * (j + 1)],
                                        mybir.ActivationFunctionType.Exp,
                                        scale=0.125,
                                    )
                                    nc.vector.tensor_mul(
                                        pt[:, 512 * j + off:512 * j + off + 128],
                                        pt[:, 512 * j + off:512 * j + off + 128],
                                        tri_sb[:],
                                    )
                            else:
                                nc.scalar.activation(
                                    pt[:],
                                    s_ps[:],
                                    mybir.ActivationFunctionType.Exp,
                                    scale=0.125,
                                )
                            for j in range(4):
                                kt = 4 * g + j
                                off = 128 * j if diag else 0
                                nc.tensor.matmul(
                                    y_ps[:, off:512],
                                    v_sb[:, kt, h * 65:(h + 1) * 65],
                                    pt[:, 512 * j + off:512 * (j + 1)],
                                    start=(kt == 0),
                                    stop=(kt == n_kt - 1),
                                )
                        # normalize: row 64 of y_ps is the softmax denominator
                        den = den_pool.tile([1, 512], F32)
                        nc.scalar.activation(
                            den[:], y_ps[64:65, :],
                            mybir.ActivationFunctionType.Identity,
                        )
                        rden = den_pool.tile([1, 512], F32)
                        nc.vector.reciprocal_approx_fast(rden[:], den[:])
                        idx = h * QB + qb
                        nc.sync.dma_start(rden_dram[idx:idx + 1, :], rden[:])
                        bc = bc_pool.tile([64, 512], F32)
                        nc.sync.dma_start(
                            bc[:], rden_dram[idx:idx + 1, :].broadcast_to([64, 512])
                        )
                        if sub == 0:
                            nc.vector.tensor_mul(
                                ynT_sb[0:64, pair, qb * 512:(qb + 1) * 512],
                                y_ps[0:64, :],
                                bc[:],
                            )
                        else:
                            tmp = shift_pool.tile([64, 512], F32R)
                            nc.vector.tensor_mul(tmp[:], y_ps[0:64, :], bc[:])
                            nc.sync.dma_start(
                                ynT_sb[64:128, pair, qb * 512:(qb + 1) * 512],
                                tmp[:],
                            )

                # ================= phase 3: projection =================
                for ti in range(TT):
                    o_t = out_pool.tile([128, C], F32)
                    for nb in range(2):
                        pp = ps_pr.tile([128, 384], F32)
                        for kc in range(CG // 128):
                            nc.tensor.matmul(
                                pp[:],
                                ynT_sb[:, kc, ti * 128:(ti + 1) * 128],
                                wp_sb[:, kc, nb * 384:(nb + 1) * 384],
                                start=(kc == 0),
                                stop=False,
                            )
                        nc.tensor.matmul(
                            pp[:], ones_sb[:], bp_sb[:, nb * 384:(nb + 1) * 384],
                            start=False, stop=True,
                        )
                        nc.vector.tensor_copy(
                            o_t[:, nb * 384:(nb + 1) * 384], pp[:]
                        )
                    nc.sync.dma_start(out_ap[ti * 128:(ti + 1) * 128, :], o_t[:])

    nc.compile()
    return nc


def _prep_inputs(x, w_attn, b_attn, w_proj, b_proj):
    """Build the 8 per-core input maps."""
    x = np.ascontiguousarray(np.asarray(x, dtype=np.float32))
    w_attn = np.asarray(w_attn, dtype=np.float32)
    b_attn = np.asarray(b_attn, dtype=np.float32)
    w_proj = np.asarray(w_proj, dtype=np.float32)
    b_proj = np.asarray(b_proj, dtype=np.float32)

    ident = np.eye(128, dtype=np.float32)
    ones1 = np.ones((1, 128), dtype=np.float32)
    # tri[kk, qc] = 1 if kk <= qc else 0  (valid keys at/below the diagonal)
    tri = np.triu(np.ones((128, 128), dtype=np.float32))

    in_maps = []
    for b in range(B):
        for hg in range(HG):
            qsl = slice(hg * CG, (hg + 1) * CG)
            ksl = slice(C + hg * CG, C + (hg + 1) * CG)
            vsl = slice(2 * C + hg * CG, 2 * C + (hg + 1) * CG)
            wqk = np.concatenate([w_attn[:, qsl], w_attn[:, ksl]], axis=1)
            bqk = np.concatenate([b_attn[qsl], b_attn[ksl]])
            bqk = np.ascontiguousarray(bqk.reshape(KC, 128).T)

            wv = np.zeros((C, VW), dtype=np.float32)
            bv = np.zeros((1, VW), dtype=np.float32)
            wv_part = w_attn[:, vsl]
            bv_part = b_attn[vsl]
            for h in range(HPG):
                wv[:, h * 65:h * 65 + 64] = wv_part[:, h * 64:(h + 1) * 64]
                bv[0, h * 65:h * 65 + 64] = bv_part[h * 64:(h + 1) * 64]
                bv[0, h * 65 + 64] = 1.0

            wp = np.ascontiguousarray(w_proj[hg * CG:(hg + 1) * CG, :])
            bp = (b_proj if hg == 0 else np.zeros_like(b_proj)).reshape(1, C)

            in_maps.append({
                "x": x[b],
                "wqk": np.ascontiguousarray(wqk),
                "wv": wv,
                "wp": wp,
                "bqk": bqk,
                "bv": bv,
                "bp": np.ascontiguousarray(bp),
                "ident": ident,
                "ones1": ones1,
                "tri": np.ascontiguousarray(tri),
            })
    return in_maps


def get_nc():
    if "nc" not in _CACHE:
        _CACHE["nc"] = _build()
    return _CACHE["nc"]


def kernel(x, w_attn, b_attn, w_proj, b_proj):
    nc = get_nc()
    in_maps = _prep_inputs(x, w_attn, b_attn, w_proj, b_proj)
    res = run_bass_kernel_spmd(nc, in_maps, core_ids=list(range(8)))
    out = np.empty((B, T, C), dtype=np.float32)
    for b in range(B):
        out[b] = res.results[2 * b]["out"] + res.results[2 * b + 1]["out"]
    return out


# revision 2
# speedup vs baseline: 1.2432x; 1.2432x over previous
"""Causal self-attention (B=4, T=2048, C=768, H=12) on 8 NeuronCores.

Sharding: core (b, hg) with b in 0..3, hg in 0..1 handles batch b and head
group hg (6 heads of 64 dims).  Each core computes q/k/v projections for its
head group, causal flash-style attention in transposed-score layout, and its
half of the output projection (rows hg*384..hg*384+384 of w_proj).  The host
sums the two half-projections per batch (the only cross-core reduction).

All matmuls run as float32r (reduced-precision fp32 mode, full PE rate at
N>=256, ~1e-4 relative accuracy).
"""

import numpy as np

import concourse.bass as bass
import concourse.bacc as bacc
import concourse.tile as tile
import concourse.mybir as mybir
from concourse.bass_utils import run_bass_kernel_spmd

F32 = mybir.dt.float32
F32R = mybir.dt.float32r

B, T, C = 4, 2048, 768
H = 12
D = 64
HG = 2            # head groups (cores per batch)
HPG = H // HG     # heads per group = 6
CG = C // HG      # channels per group = 384
KC = C // 128     # contraction chunks for C = 6
TT = T // 128     # t-tiles = 16
QB = T // 512     # query blocks = 4
VW = HPG * (D + 1)  # interleaved v width (ones col per head) = 390

_CACHE = {}


def _build():
    nc = bacc.Bacc("TRN2", target_bir_lowering=False, debug=False, num_devices=8)

    x_ap = nc.dram_tensor("x", [T, C], F32R, kind="ExternalInput").ap()
    wqk_ap = nc.dram_tensor("wqk", [C, 2 * CG], F32R, kind="ExternalInput").ap()
    wv_ap = nc.dram_tensor("wv", [C, VW], F32R, kind="ExternalInput").ap()
    wp_ap = nc.dram_tensor("wp", [CG, C], F32R, kind="ExternalInput").ap()
    bqk_ap = nc.dram_tensor("bqk", [128, KC], F32, kind="ExternalInput").ap()
    bv_ap = nc.dram_tensor("bv", [1, VW], F32R, kind="ExternalInput").ap()
    bp_ap = nc.dram_tensor("bp", [1, C], F32R, kind="ExternalInput").ap()
    ident_ap = nc.dram_tensor("ident", [128, 128], F32R, kind="ExternalInput").ap()
    ones_ap = nc.dram_tensor("ones1", [1, 128], F32R, kind="ExternalInput").ap()
    tri_ap = nc.dram_tensor("tri", [128, 128], F32R, kind="ExternalInput").ap()
    out_ap = nc.dram_tensor("out", [T, C], F32, kind="ExternalOutput").ap()
    rden_dram = nc.dram_tensor("rden_scratch", [HPG * QB, 512], F32).ap()

    with tile.TileContext(nc) as tc:
        import contextlib

        st = contextlib.ExitStack()
        with st:
            const = st.enter_context(tc.tile_pool(name="const", bufs=1))
            big = st.enter_context(tc.tile_pool(name="big", bufs=1))

            # ---- constants / weights (scalar-engine DMA queue, so the big
            # weight loads don't sit in front of the x tiles on the SP queue)
            wqk_sb = const.tile([128, KC, 2 * CG], F32R)
            nc.scalar.dma_start(wqk_sb[:], wqk_ap.rearrange("(k p) m -> p k m", p=128))
            wv_sb = const.tile([128, KC, VW], F32R)
            nc.scalar.dma_start(wv_sb[:], wv_ap.rearrange("(k p) m -> p k m", p=128))
            wp_sb = const.tile([128, CG // 128, C], F32R)
            nc.scalar.dma_start(wp_sb[:], wp_ap.rearrange("(k p) m -> p k m", p=128))
            bqk_sb = const.tile([128, KC], F32)
            nc.scalar.dma_start(bqk_sb[:], bqk_ap[:])
            bv_sb = const.tile([1, VW], F32R)
            nc.scalar.dma_start(bv_sb[:], bv_ap[:])
            bp_sb = const.tile([1, C], F32R)
            nc.scalar.dma_start(bp_sb[:], bp_ap[:])
            ident_sb = const.tile([128, 128], F32R)
            nc.sync.dma_start(ident_sb[:], ident_ap[:])
            ones_sb = const.tile([1, 128], F32R)
            nc.sync.dma_start(ones_sb[:], ones_ap[:])
            tri_sb = const.tile([128, 128], F32R)
            nc.sync.dma_start(tri_sb[:], tri_ap[:])

            # ---- persistent activations
            qkT_sb = big.tile([128, 2 * CG // 128, T], F32R)   # chunks 0-2: q pairs, 3-5: k pairs
            v_sb = big.tile([128, TT, VW], F32R)               # interleaved v + ones cols
            ynT_sb = big.tile([128, CG // 128, T], F32R)       # normalized y^T

            # ================= phase 1: x^T + qkv =================
            # tb-major so attention over early key/query blocks can start
            # while later blocks are still projecting.
            with tc.tile_pool(name="xin", bufs=3) as xin_pool, \
                 tc.tile_pool(name="xT", bufs=1) as xT_pool, \
                 tc.tile_pool(name="ps_t", bufs=3, space="PSUM") as ps_t, \
                 tc.tile_pool(name="ps_qk", bufs=2, space="PSUM") as ps_qk, \
                 tc.tile_pool(name="ps_v", bufs=2, space="PSUM") as ps_v:

                xT_sb = xT_pool.tile([128, KC, T], F32R)
                for tb in range(QB):
                    for ti in range(4 * tb, 4 * tb + 4):
                        x_t = xin_pool.tile([128, C], F32R)
                        nc.sync.dma_start(x_t[:], x_ap[ti * 128:(ti + 1) * 128, :])
                        for cc in range(KC):
                            pst = ps_t.tile([128, 128], F32)
                            nc.tensor.transpose(
                                pst[:].bitcast(F32R),
                                x_t[:, cc * 128:(cc + 1) * 128],
                                ident_sb[:],
                            )
                            nc.vector.tensor_copy(
                                xT_sb[:, cc, ti * 128:(ti + 1) * 128], pst[:]
                            )

                    # q^T / k^T chunks for this t-block
                    for m in range(2 * CG // 128):
                        pqk = ps_qk.tile([128, 512], F32)
                        for kc in range(KC):
                            nc.tensor.matmul(
                                pqk[:],
                                wqk_sb[:, kc, m * 128:(m + 1) * 128],
                                xT_sb[:, kc, tb * 512:(tb + 1) * 512],
                                start=(kc == 0),
                                stop=(kc == KC - 1),
                            )
                        nc.vector.tensor_scalar_add(
                            qkT_sb[:, m, tb * 512:(tb + 1) * 512],
                            pqk[:],
                            bqk_sb[:, m:m + 1],
                        )

                    # v tiles for this t-block (interleaved + ones cols)
                    for ti in range(4 * tb, 4 * tb + 4):
                        pv = ps_v.tile([128, VW], F32)
                        for kc in range(KC):
                            nc.tensor.matmul(
                                pv[:],
                                xT_sb[:, kc, ti * 128:(ti + 1) * 128],
                                wv_sb[:, kc, :],
                                start=(kc == 0),
                                stop=False,
                            )
                        nc.tensor.matmul(
                            pv[:], ones_sb[:], bv_sb[:], start=False, stop=True
                        )
                        nc.vector.tensor_copy(v_sb[:, ti, :], pv[:])

            # ================= phase 2: attention =================
            with tc.tile_pool(name="pt", bufs=4) as pt_pool, \
                 tc.tile_pool(name="den", bufs=4) as den_pool, \
                 tc.tile_pool(name="bc", bufs=3) as bc_pool, \
                 tc.tile_pool(name="shift", bufs=2) as shift_pool, \
                 tc.tile_pool(name="ps_s", bufs=2, space="PSUM") as ps_s, \
                 tc.tile_pool(name="ps_av", bufs=2, space="PSUM") as ps_av, \
                 tc.tile_pool(name="ps_pr", bufs=2, space="PSUM") as ps_pr, \
                 tc.tile_pool(name="out_sb", bufs=2) as out_pool:

                for qb in range(QB):
                    for h in range(HPG):
                        pair, sub = h // 2, h % 2
                        p0 = 64 * sub
                        y_ps = ps_av.tile([65, 512], F32)
                        n_kt = 4 * (qb + 1)
                        for g in range(n_kt // 2):
                            s_ps = ps_s.tile([128, 1024], F32)
                            pt = pt_pool.tile([128, 1024], F32R)
                            kts = (2 * g, 2 * g + 1)
                            offs = [max(0, 128 * (kt - 4 * qb)) for kt in kts]
                            for j, (kt, off) in enumerate(zip(kts, offs)):
                                nc.tensor.matmul(
                                    s_ps[:, 512 * j + off:512 * (j + 1)],
                                    qkT_sb[p0:p0 + 64, 3 + pair,
                                           kt * 128:(kt + 1) * 128],
                                    qkT_sb[p0:p0 + 64, pair,
                                           qb * 512 + off:(qb + 1) * 512],
                                    start=True,
                                    stop=True,
                                )
                            if kts[1] >= 4 * qb:  # group touches the diagonal band
                                for j, (kt, off) in enumerate(zip(kts, offs)):
                                    nc.scalar.activation(
                                        pt[:, 512 * j + off:512 * (j + 1)],
                                        s_ps[:, 512 * j + off:512 * (j + 1)],
                                        mybir.ActivationFunctionType.Exp,
                                        scale=0.125,
                                    )
                                    if kt >= 4 * qb:
                                        nc.vector.tensor_mul(
                                            pt[:, 512 * j + off:512 * j + off + 128],
                                            pt[:, 512 * j + off:512 * j + off + 128],
                                            tri_sb[:],
                                        )
                            else:
                                nc.scalar.activation(
                                    pt[:],
                                    s_ps[:],
                                    mybir.ActivationFunctionType.Exp,
                                    scale=0.125,
                                )
                            for j, (kt, off) in enumerate(zip(kts, offs)):
                                nc.tensor.matmul(
                                    y_ps[:, off:512],
                                    v_sb[:, kt, h * 65:(h + 1) * 65],
                                    pt[:, 512 * j + off:512 * (j + 1)],
                                    start=(kt == 0),
                                    stop=(kt == n_kt - 1),
                                )
                        # normalize: row 64 of y_ps is the softmax denominator
                        den = den_pool.tile([1, 512], F32)
                        nc.scalar.activation(
                            den[:], y_ps[64:65, :],
                            mybir.ActivationFunctionType.Identity,
                        )
                        rden = den_pool.tile([1, 512], F32)
                        nc.vector.reciprocal_approx_fast(rden[:], den[:])
                        idx = h * QB + qb
                        nc.sync.dma_start(rden_dram[idx:idx + 1, :], rden[:])
                        bc = bc_pool.tile([64, 512], F32)
                        nc.sync.dma_start(
                            bc[:], rden_dram[idx:idx + 1, :].broadcast_to([64, 512])
                        )
                        if sub == 0:
                            nc.vector.tensor_mul(
                                ynT_sb[0:64, pair, qb * 512:(qb + 1) * 512],
                                y_ps[0:64, :],
                                bc[:],
                            )
                        else:
                            tmp = shift_pool.tile([64, 512], F32R)
                            nc.vector.tensor_mul(tmp[:], y_ps[0:64, :], bc[:])
                            nc.sync.dma_start(
                                ynT_sb[64:128, pair, qb * 512:(qb + 1) * 512],
                                tmp[:],
                            )

                    # ============ projection for this query block ============
                    for ti in range(4 * qb, 4 * qb + 4):
                        o_t = out_pool.tile([128, C], F32)
                        for nb in range(2):
                            pp = ps_pr.tile([128, 384], F32)
                            for kc in range(CG // 128):
                                nc.tensor.matmul(
                                    pp[:],
                                    ynT_sb[:, kc, ti * 128:(ti + 1) * 128],
                                    wp_sb[:, kc, nb * 384:(nb + 1) * 384],
                                    start=(kc == 0),
                                    stop=False,
                                )
                            nc.tensor.matmul(
                                pp[:], ones_sb[:], bp_sb[:, nb * 384:(nb + 1) * 384],
                                start=False, stop=True,
                            )
                            nc.vector.tensor_copy(
                                o_t[:, nb * 384:(nb + 1) * 384], pp[:]
                            )
                        nc.sync.dma_start(out_ap[ti * 128:(ti + 1) * 128, :], o_t[:])

    nc.compile()
    return nc


def _prep_inputs(x, w_attn, b_attn, w_proj, b_proj):
    """Build the 8 per-core input maps."""
    x = np.ascontiguousarray(np.asarray(x, dtype=np.float32))
    w_attn = np.asarray(w_attn, dtype=np.float32)
    b_attn = np.asarray(b_attn, dtype=np.float32)
    w_proj = np.asarray(w_proj, dtype=np.float32)
    b_proj = np.asarray(b_proj, dtype=np.float32)

    ident = np.eye(128, dtype=np.float32)
    ones1 = np.ones((1, 128), dtype=np.float32)
    # tri[kk, qc] = 1 if kk <= qc else 0  (valid keys at/below the diagonal)
    tri = np.triu(np.ones((128, 128), dtype=np.float32))

    in_maps = []
    for b in range(B):
        for hg in range(HG):
            qsl = slice(hg * CG, (hg + 1) * CG)
            ksl = slice(C + hg * CG, C + (hg + 1) * CG)
            vsl = slice(2 * C + hg * CG, 2 * C + (hg + 1) * CG)
            wqk = np.concatenate([w_attn[:, qsl], w_attn[:, ksl]], axis=1)
            bqk = np.concatenate([b_attn[qsl], b_attn[ksl]])
            bqk = np.ascontiguousarray(bqk.reshape(KC, 128).T)

            wv = np.zeros((C, VW), dtype=np.float32)
            bv = np.zeros((1, VW), dtype=np.float32)
            wv_part = w_attn[:, vsl]
            bv_part = b_attn[vsl]
            for h in range(HPG):
                wv[:, h * 65:h * 65 + 64] = wv_part[:, h * 64:(h + 1) * 64]
                bv[0, h * 65:h * 65 + 64] = bv_part[h * 64:(h + 1) * 64]
                bv[0, h * 65 + 64] = 1.0

            wp = np.ascontiguousarray(w_proj[hg * CG:(hg + 1) * CG, :])
            bp = (b_proj if hg == 0 else np.zeros_like(b_proj)).reshape(1, C)

            in_maps.append({
                "x": x[b],
                "wqk": np.ascontiguousarray(wqk),
                "wv": wv,
                "wp": wp,
                "bqk": bqk,
                "bv": bv,
                "bp": np.ascontiguousarray(bp),
                "ident": ident,
                "ones1": ones1,
                "tri": np.ascontiguousarray(tri),
            })
    return in_maps


def get_nc():
    if "nc" not in _CACHE:
        _CACHE["nc"] = _build()
    return _CACHE["nc"]


def kernel(x, w_attn, b_attn, w_proj, b_proj):
    nc = get_nc()
    in_maps = _prep_inputs(x, w_attn, b_attn, w_proj, b_proj)
    res = run_bass_kernel_spmd(nc, in_maps, core_ids=list(range(8)))
    out = np.empty((B, T, C), dtype=np.float32)
    for b in range(B):
        out[b] = res.results[2 * b]["out"] + res.results[2 * b + 1]["out"]
    return out


# revision 5
# speedup vs baseline: 1.2434x; 1.0002x over previous
"""Causal self-attention (B=4, T=2048, C=768, H=12) on 8 NeuronCores.

Sharding: core (b, hg) with b in 0..3, hg in 0..1 handles batch b and head
group hg (6 heads of 64 dims).  Each core computes q/k/v projections for its
head group, causal flash-style attention in transposed-score layout, and its
half of the output projection (rows hg*384..hg*384+384 of w_proj).  The host
sums the two half-projections per batch (the only cross-core reduction).

All matmuls run as float32r (reduced-precision fp32 mode, full PE rate at
N>=256, ~1e-4 relative accuracy).
"""

import numpy as np

import concourse.bass as bass
import concourse.bacc as bacc
import concourse.tile as tile
import concourse.mybir as mybir
from concourse.bass_utils import run_bass_kernel_spmd

F32 = mybir.dt.float32
F32R = mybir.dt.float32r

B, T, C = 4, 2048, 768
H = 12
D = 64
HG = 2            # head groups (cores per batch)
HPG = H // HG     # heads per group = 6
CG = C // HG      # channels per group = 384
KC = C // 128     # contraction chunks for C = 6
TT = T // 128     # t-tiles = 16
QB = T // 512     # query blocks = 4
VW = HPG * (D + 1)  # interleaved v width (ones col per head) = 390

_CACHE = {}


def _build():
    nc = bacc.Bacc("TRN2", target_bir_lowering=False, debug=False, num_devices=8)

    x_ap = nc.dram_tensor("x", [T, C], F32R, kind="ExternalInput").ap()
    wqk_ap = nc.dram_tensor("wqk", [C, 2 * CG], F32R, kind="ExternalInput").ap()
    wv_ap = nc.dram_tensor("wv", [C, VW], F32R, kind="ExternalInput").ap()
    wp_ap = nc.dram_tensor("wp", [CG, C], F32R, kind="ExternalInput").ap()
    bqk_ap = nc.dram_tensor("bqk", [128, KC], F32, kind="ExternalInput").ap()
    bv_ap = nc.dram_tensor("bv", [1, VW], F32R, kind="ExternalInput").ap()
    bp_ap = nc.dram_tensor("bp", [1, C], F32R, kind="ExternalInput").ap()
    ident_ap = nc.dram_tensor("ident", [128, 128], F32R, kind="ExternalInput").ap()
    ones_ap = nc.dram_tensor("ones1", [1, 128], F32R, kind="ExternalInput").ap()
    tri_ap = nc.dram_tensor("tri", [128, 128], F32R, kind="ExternalInput").ap()
    out_ap = nc.dram_tensor("out", [T, C], F32, kind="ExternalOutput").ap()
    rden_dram = nc.dram_tensor("rden_scratch", [HPG * QB, 512], F32).ap()

    with tile.TileContext(nc) as tc:
        import contextlib

        st = contextlib.ExitStack()
        with st:
            const = st.enter_context(tc.tile_pool(name="const", bufs=1))
            big = st.enter_context(tc.tile_pool(name="big", bufs=1))

            # ---- constants / weights (scalar-engine DMA queue, so the big
            # weight loads don't sit in front of the x tiles on the SP queue)
            wqk_sb = const.tile([128, KC, 2 * CG], F32R)
            nc.scalar.dma_start(wqk_sb[:], wqk_ap.rearrange("(k p) m -> p k m", p=128))
            wv_sb = const.tile([128, KC, VW], F32R)
            nc.scalar.dma_start(wv_sb[:], wv_ap.rearrange("(k p) m -> p k m", p=128))
            wp_sb = const.tile([128, CG // 128, C], F32R)
            nc.scalar.dma_start(wp_sb[:], wp_ap.rearrange("(k p) m -> p k m", p=128))
            bqk_sb = const.tile([128, KC], F32)
            nc.scalar.dma_start(bqk_sb[:], bqk_ap[:])
            bv_sb = const.tile([1, VW], F32R)
            nc.scalar.dma_start(bv_sb[:], bv_ap[:])
            bp_sb = const.tile([1, C], F32R)
            nc.scalar.dma_start(bp_sb[:], bp_ap[:])
            ident_sb = const.tile([128, 128], F32R)
            nc.sync.dma_start(ident_sb[:], ident_ap[:])
            ones_sb = const.tile([1, 128], F32R)
            nc.sync.dma_start(ones_sb[:], ones_ap[:])
            tri_sb = const.tile([128, 128], F32R)
            nc.sync.dma_start(tri_sb[:], tri_ap[:])

            # ---- persistent activations
            qkT_sb = big.tile([128, 2 * CG // 128, T], F32R)   # chunks 0-2: q pairs, 3-5: k pairs
            v_sb = big.tile([128, TT, VW], F32R)               # interleaved v + ones cols
            ynT_sb = big.tile([128, CG // 128, T], F32R)       # normalized y^T

            # ================= phase 1: x^T + qkv =================
            # tb-major so attention over early key/query blocks can start
            # while later blocks are still projecting.
            with tc.tile_pool(name="xin", bufs=3) as xin_pool, \
                 tc.tile_pool(name="xT", bufs=1) as xT_pool, \
                 tc.tile_pool(name="ps_t", bufs=3, space="PSUM") as ps_t, \
                 tc.tile_pool(name="ps_qk", bufs=2, space="PSUM") as ps_qk, \
                 tc.tile_pool(name="ps_v", bufs=2, space="PSUM") as ps_v:

                xT_sb = xT_pool.tile([128, KC, T], F32R)
                for tb in range(QB):
                    for ti in range(4 * tb, 4 * tb + 4):
                        x_t = xin_pool.tile([128, C], F32R)
                        nc.sync.dma_start(x_t[:], x_ap[ti * 128:(ti + 1) * 128, :])
                        for cc in range(KC):
                            pst = ps_t.tile([128, 128], F32)
                            nc.tensor.transpose(
                                pst[:].bitcast(F32R),
                                x_t[:, cc * 128:(cc + 1) * 128],
                                ident_sb[:],
                            )
                            nc.vector.tensor_copy(
                                xT_sb[:, cc, ti * 128:(ti + 1) * 128], pst[:]
                            )

                    # q^T / k^T chunks for this t-block
                    for m in range(2 * CG // 128):
                        pqk = ps_qk.tile([128, 512], F32)
                        for kc in range(KC):
                            nc.tensor.matmul(
                                pqk[:],
                                wqk_sb[:, kc, m * 128:(m + 1) * 128],
                                xT_sb[:, kc, tb * 512:(tb + 1) * 512],
                                start=(kc == 0),
                                stop=(kc == KC - 1),
                            )
                        nc.vector.tensor_scalar_add(
                            qkT_sb[:, m, tb * 512:(tb + 1) * 512],
                            pqk[:],
                            bqk_sb[:, m:m + 1],
                        )

                    # v tiles for this t-block (interleaved + ones cols)
                    for ti in range(4 * tb, 4 * tb + 4):
                        pv = ps_v.tile([128, VW], F32)
                        for kc in range(KC):
                            nc.tensor.matmul(
                                pv[:],
                                xT_sb[:, kc, ti * 128:(ti + 1) * 128],
                                wv_sb[:, kc, :],
                                start=(kc == 0),
                                stop=False,
                            )
                        nc.tensor.matmul(
                            pv[:], ones_sb[:], bv_sb[:], start=False, stop=True
                        )
                        nc.vector.tensor_copy(v_sb[:, ti, :], pv[:])

            # ================= phase 2: attention =================
            with tc.tile_pool(name="pt", bufs=4) as pt_pool, \
                 tc.tile_pool(name="den", bufs=6) as den_pool, \
                 tc.tile_pool(name="bc", bufs=4) as bc_pool, \
                 tc.tile_pool(name="shift", bufs=2) as shift_pool, \
                 tc.tile_pool(name="ps_s", bufs=2, space="PSUM") as ps_s, \
                 tc.tile_pool(name="ps_av", bufs=2, space="PSUM") as ps_av, \
                 tc.tile_pool(name="ps_pr", bufs=2, space="PSUM") as ps_pr, \
                 tc.tile_pool(name="out_sb", bufs=2) as out_pool:

                for qb in range(QB):
                    for h in range(HPG):
                        pair, sub = h // 2, h % 2
                        p0 = 64 * sub
                        y_ps = ps_av.tile([65, 512], F32, tag="y_ps")
                        n_kt = 4 * (qb + 1)
                        for g in range(n_kt // 2):
                            s_ps = ps_s.tile([128, 1024], F32)
                            pt = pt_pool.tile([128, 1024], F32R)
                            kts = (2 * g, 2 * g + 1)
                            offs = [max(0, 128 * (kt - 4 * qb)) for kt in kts]
                            for j, (kt, off) in enumerate(zip(kts, offs)):
                                nc.tensor.matmul(
                                    s_ps[:, 512 * j + off:512 * (j + 1)],
                                    qkT_sb[p0:p0 + 64, 3 + pair,
                                           kt * 128:(kt + 1) * 128],
                                    qkT_sb[p0:p0 + 64, pair,
                                           qb * 512 + off:(qb + 1) * 512],
                                    start=True,
                                    stop=True,
                                )
                            if kts[1] >= 4 * qb:  # group touches the diagonal band
                                for j, (kt, off) in enumerate(zip(kts, offs)):
                                    nc.scalar.activation(
                                        pt[:, 512 * j + off:512 * (j + 1)],
                                        s_ps[:, 512 * j + off:512 * (j + 1)],
                                        mybir.ActivationFunctionType.Exp,
                                        scale=0.125,
                                    )
                                    if kt >= 4 * qb:
                                        nc.vector.tensor_mul(
                                            pt[:, 512 * j + off:512 * j + off + 128],
                                            pt[:, 512 * j + off:512 * j + off + 128],
                                            tri_sb[:],
                                        )
                            else:
                                nc.scalar.activation(
                                    pt[:],
                                    s_ps[:],
                                    mybir.ActivationFunctionType.Exp,
                                    scale=0.125,
                                )
                            for j, (kt, off) in enumerate(zip(kts, offs)):
                                nc.tensor.matmul(
                                    y_ps[:, off:512],
                                    v_sb[:, kt, h * 65:(h + 1) * 65],
                                    pt[:, 512 * j + off:512 * (j + 1)],
                                    start=(kt == 0),
                                    stop=(kt == n_kt - 1),
                                )
                        # normalize: row 64 of y_ps is the softmax denominator
                        den = den_pool.tile([1, 512], F32)
                        nc.scalar.activation(
                            den[:], y_ps[64:65, :],
                            mybir.ActivationFunctionType.Identity,
                        )
                        rden = den_pool.tile([1, 512], F32)
                        nc.vector.reciprocal_approx_fast(rden[:], den[:])
                        idx = h * QB + qb
                        nc.sync.dma_start(rden_dram[idx:idx + 1, :], rden[:])
                        bc = bc_pool.tile([64, 512], F32)
                        nc.sync.dma_start(
                            bc[:], rden_dram[idx:idx + 1, :].broadcast_to([64, 512])
                        )
                        if sub == 0:
                            nc.vector.tensor_mul(
                                ynT_sb[0:64, pair, qb * 512:(qb + 1) * 512],
                                y_ps[0:64, :],
                                bc[:],
                            )
                        else:
                            tmp = shift_pool.tile([64, 512], F32R)
                            nc.vector.tensor_mul(tmp[:], y_ps[0:64, :], bc[:])
                            nc.sync.dma_start(
                                ynT_sb[64:128, pair, qb * 512:(qb + 1) * 512],
                                tmp[:],
                            )

                    # ============ projection for this query block ============
                    for ti in range(4 * qb, 4 * qb + 4):
                        o_t = out_pool.tile([128, C], F32)
                        for nb in range(2):
                            pp = ps_pr.tile([128, 384], F32)
                            for kc in range(CG // 128):
                                nc.tensor.matmul(
                                    pp[:],
                                    ynT_sb[:, kc, ti * 128:(ti + 1) * 128],
                                    wp_sb[:, kc, nb * 384:(nb + 1) * 384],
                                    start=(kc == 0),
                                    stop=False,
                                )
                            nc.tensor.matmul(
                                pp[:], ones_sb[:], bp_sb[:, nb * 384:(nb + 1) * 384],
                                start=False, stop=True,
                            )
                            nc.vector.tensor_copy(
                                o_t[:, nb * 384:(nb + 1) * 384], pp[:]
                            )
                        nc.sync.dma_start(out_ap[ti * 128:(ti + 1) * 128, :], o_t[:])

    nc.compile()
    return nc


def _prep_inputs(x, w_attn, b_attn, w_proj, b_proj):
    """Build the 8 per-core input maps."""
    x = np.ascontiguousarray(np.asarray(x, dtype=np.float32))
    w_attn = np.asarray(w_attn, dtype=np.float32)
    b_attn = np.asarray(b_attn, dtype=np.float32)
    w_proj = np.asarray(w_proj, dtype=np.float32)
    b_proj = np.asarray(b_proj, dtype=np.float32)

    ident = np.eye(128, dtype=np.float32)
    ones1 = np.ones((1, 128), dtype=np.float32)
    # tri[kk, qc] = 1 if kk <= qc else 0  (valid keys at/below the diagonal)
    tri = np.triu(np.ones((128, 128), dtype=np.float32))

    in_maps = []
    for b in range(B):
        for hg in range(HG):
            qsl = slice(hg * CG, (hg + 1) * CG)
            ksl = slice(C + hg * CG, C + (hg + 1) * CG)
            vsl = slice(2 * C + hg * CG, 2 * C + (hg + 1) * CG)
            wqk = np.concatenate([w_attn[:, qsl], w_attn[:, ksl]], axis=1)
            bqk = np.concatenate([b_attn[qsl], b_attn[ksl]])
            bqk = np.ascontiguousarray(bqk.reshape(KC, 128).T)

            wv = np.zeros((C, VW), dtype=np.float32)
            bv = np.zeros((1, VW), dtype=np.float32)
            wv_part = w_attn[:, vsl]
            bv_part = b_attn[vsl]
            for h in range(HPG):
                wv[:, h * 65:h * 65 + 64] = wv_part[:, h * 64:(h + 1) * 64]
                bv[0, h * 65:h * 65 + 64] = bv_part[h * 64:(h + 1) * 64]
                bv[0, h * 65 + 64] = 1.0

            wp = np.ascontiguousarray(w_proj[hg * CG:(hg + 1) * CG, :])
            bp = (b_proj if hg == 0 else np.zeros_like(b_proj)).reshape(1, C)

            in_maps.append({
                "x": x[b],
                "wqk": np.ascontiguousarray(wqk),
                "wv": wv,
                "wp": wp,
                "bqk": bqk,
                "bv": bv,
                "bp": np.ascontiguousarray(bp),
                "ident": ident,
                "ones1": ones1,
                "tri": np.ascontiguousarray(tri),
            })
    return in_maps


def get_nc():
    if "nc" not in _CACHE:
        _CACHE["nc"] = _build()
    return _CACHE["nc"]


def kernel(x, w_attn, b_attn, w_proj, b_proj):
    nc = get_nc()
    in_maps = _prep_inputs(x, w_attn, b_attn, w_proj, b_proj)
    res = run_bass_kernel_spmd(nc, in_maps, core_ids=list(range(8)))
    out = np.empty((B, T, C), dtype=np.float32)
    for b in range(B):
        out[b] = res.results[2 * b]["out"] + res.results[2 * b + 1]["out"]
    return out
